# revision 8
# baseline (speedup 1.0000x reference)
"""Bass/Trainium2 SPMD kernel for nn_MultiHeadAttention (B=2, S=2048, D=1024, H=16).

Sharding: 8 cores; core c handles batch b = c % 2 and 4 heads g = c // 2
(heads 4g..4g+3).  Each core computes its heads' Q/K/V projections, causal
softmax(QK^T/sqrt(dk) + pos_bias) with the mask folded into pos_bias on the
host (-1e30 above the diagonal), the attn output slice, and a partial output
projection (summed across head-groups on the host).

Device dataflow (all matmuls in float32r - full-speed fp32, ~1.5e-4 rel err):
  - host pre-transposes x so x^T loads are contiguous; weights pre-sliced,
    1/sqrt(dk) folded into Wq
  - projections produce Q^T,K^T [j, s] and V [s, j] directly on PE
  - scores: pos_bias loaded to PSUM via identity-matmul, QK^T accumulates
  - softmax: exp pass 1 on ScalarE with fused row-sum (accum_out), then
    pass 2 recomputes exp(score - ln(rowsum)) => normalized attn, no DVE
    reciprocal/reduce needed (those ops are broken on this runtime)
  - attn tiles PE-transposed for the PV matmul; context^T @ Wo^T partial out
"""
import numpy as np

B, S, D, H = 2, 2048, 1024, 16
DK = 64
NCORES = 8
HPC = 4  # heads per core

_nc_cache = {}


def _build(S, D, HPC, write_zeros=True):
    import concourse.bacc as bacc
    import concourse.mybir as mybir
    import concourse.tile as tile

    F32 = mybir.dt.float32
    F32R = mybir.dt.float32r
    AF = mybir.ActivationFunctionType

    JW = HPC * DK          # per-core j width (256)
    NJT = max(1, JW // 128)  # j partition-tiles (2)
    ND = D // 128          # d tiles (8)
    NS = S // 128          # s tiles (16)
    NG = NS // 4           # q groups of 4 tiles (4)
    NO = D // 512          # out col blocks (2)

    nc = bacc.Bacc("TRN2", target_bir_lowering=False, debug=False)

    xqT = nc.dram_tensor("xqT", [D, S], F32R, kind="ExternalInput").ap()
    xkT = nc.dram_tensor("xkT", [D, S], F32R, kind="ExternalInput").ap()
    xvT = nc.dram_tensor("xvT", [D, S], F32R, kind="ExternalInput").ap()
    wqT = nc.dram_tensor("wqT", [D, JW], F32R, kind="ExternalInput").ap()
    wkT = nc.dram_tensor("wkT", [D, JW], F32R, kind="ExternalInput").ap()
    wvT = nc.dram_tensor("wvT", [D, JW], F32R, kind="ExternalInput").ap()
    woT = nc.dram_tensor("woT", [JW, D], F32R, kind="ExternalInput").ap()
    pb = nc.dram_tensor("pb", [HPC, S, S], F32R, kind="ExternalInput").ap()
    identD = nc.dram_tensor("ident", [128, 128], F32R, kind="ExternalInput").ap()

    attn_out = nc.dram_tensor("attn_out", [HPC, S, S], F32, kind="ExternalOutput").ap()
    pout = nc.dram_tensor("pout", [S, D], F32, kind="ExternalOutput").ap()

    with tile.TileContext(nc) as tc:
        with (
            tc.tile_pool(name="const", bufs=1) as cpool,
            tc.tile_pool(name="qkv", bufs=1) as qkv,
        ):
            ident = cpool.tile([128, 128], F32R)
            nc.sync.dma_start(ident[:], identD[:])
            # wo laid per-head on partitions 0..63: col h*D + o
            wo_sb = cpool.tile([128, HPC * D], F32R)
            for h in range(HPC):
                nc.sync.dma_start(wo_sb[0:64, h * D:(h + 1) * D], woT[h * 64:(h + 1) * 64, :])
            zero_sb = cpool.tile([128, S], F32)
            nc.vector.memset(zero_sb[:], 0.0)

            # upper-triangle zeros of attn, written up front (overlap proj DMA)
            if write_zeros:
                for h in range(HPC):
                    for qi in range(NS - 1):
                        W = (qi + 1) * 128
                        nc.sync.dma_start(
                            attn_out[h, qi * 128:(qi + 1) * 128, W:S],
                            zero_sb[:, 0:S - W],
                        )

            QT_sb = qkv.tile([128, NJT * S], F32R)
            KT_sb = qkv.tile([128, NJT * S], F32R)
            V_sb = qkv.tile([128, NS * JW], F32R)
            # per-head ctx^T on partitions 0..63: col h*S + s
            ctxT_sb = qkv.tile([128, HPC * S], F32R)

            # ---------------- projections ----------------
            with (
                tc.tile_pool(name="xt", bufs=1) as xtp,
                tc.tile_pool(name="wts", bufs=1) as wts,
                tc.tile_pool(name="pp", bufs=1, space="PSUM") as pp,
            ):
                wq_sb = wts.tile([128, ND * JW], F32R)
                wk_sb = wts.tile([128, ND * JW], F32R)
                wv_sb = wts.tile([128, ND * JW], F32R)
                for d in range(ND):
                    nc.sync.dma_start(wq_sb[:, d * JW:(d + 1) * JW], wqT[d * 128:(d + 1) * 128, :])
                    nc.sync.dma_start(wk_sb[:, d * JW:(d + 1) * JW], wkT[d * 128:(d + 1) * 128, :])
                    nc.sync.dma_start(wv_sb[:, d * JW:(d + 1) * JW], wvT[d * 128:(d + 1) * 128, :])
                with nc.named_scope("proj_qk"):
                    for (xT, w_sb, O_sb) in ((xqT, wq_sb, QT_sb), (xkT, wk_sb, KT_sb)):
                        for sb4 in range(S // 512):
                            psums = [pp.tile([128, 512], F32, tag="pj", bufs=6, name=f"pq{_i}") for _i in range(NJT)]
                            for d in range(ND):
                                xt = xtp.tile([128, 512], F32R, tag="xt", bufs=3)
                                nc.sync.dma_start(xt[:], xT[d * 128:(d + 1) * 128, sb4 * 512:(sb4 + 1) * 512])
                                for jt in range(NJT):
                                    nc.tensor.matmul(
                                        psums[jt][:],
                                        w_sb[:, d * JW + jt * 128: d * JW + (jt + 1) * 128],
                                        xt[:],
                                        start=(d == 0),
                                        stop=(d == ND - 1),
                                    )
                            for jt in range(NJT):
                                nc.scalar.copy(O_sb[:, jt * S + sb4 * 512: jt * S + (sb4 + 1) * 512], psums[jt][:])
                with nc.named_scope("proj_v"):
                    for sb4 in range(S // 512):
                        psums = [pp.tile([128, 512], F32, tag="pj", bufs=6, name=f"pv{_i}") for _i in range(4)]
                        for d in range(ND):
                            xt = xtp.tile([128, 512], F32R, tag="xt", bufs=3)
                            nc.sync.dma_start(xt[:], xvT[d * 128:(d + 1) * 128, sb4 * 512:(sb4 + 1) * 512])
                            for st in range(4):
                                nc.tensor.matmul(
                                    psums[st][:, 0:JW],
                                    xt[:, st * 128:(st + 1) * 128],
                                    wv_sb[:, d * JW:(d + 1) * JW],
                                    start=(d == 0),
                                    stop=(d == ND - 1),
                                )
                        for st in range(4):
                            nc.scalar.copy(
                                V_sb[:, (sb4 * 4 + st) * JW:(sb4 * 4 + st + 1) * JW],
                                psums[st][:, 0:JW],
                            )

            # ---------------- attention ----------------
            def qt_slice(h, qi):
                return QT_sb[(h % 2) * 64:(h % 2) * 64 + 64,
                             (h // 2) * S + qi * 128:(h // 2) * S + (qi + 1) * 128]

            def kt_slice(h, k0, w):
                return KT_sb[(h % 2) * 64:(h % 2) * 64 + 64,
                             (h // 2) * S + k0:(h // 2) * S + k0 + w]

            def v_slice(kt, h):
                return V_sb[:, kt * JW + h * DK: kt * JW + h * DK + DK]

            with (
                tc.tile_pool(name="att", bufs=1) as attp,
                tc.tile_pool(name="smalls", bufs=1) as smalls,
                tc.tile_pool(name="ps_s", bufs=1, space="PSUM") as ps_s,
                tc.tile_pool(name="ps_t", bufs=1, space="PSUM") as ps_t,
                tc.tile_pool(name="ps_c", bufs=1, space="PSUM") as ps_c,
            ):
                with nc.named_scope("attention"):
                    for h in range(HPC):
                        hrow = (h % 2) * 64
                        for gq in range(NG):
                            group_attn = []
                            for qi4 in range(4):
                                qi = gq * 4 + qi4
                                W = (qi + 1) * 128
                                nch = (W + 1023) // 1024
                                bias_t = attp.tile([128, S], F32R, tag="bias", bufs=2)
                                nc.sync.dma_start(
                                    bias_t[:, 0:W], pb[h, qi * 128:(qi + 1) * 128, 0:W]
                                )
                                att_t = attp.tile([128, S], F32R, tag="attn", bufs=6)
                                dpart = smalls.tile([128, 2], F32, tag="dpart", bufs=4)
                                pscores = []
                                for ch in range(nch):
                                    base = ch * 1024
                                    wch = min(1024, W - base)
                                    ps = ps_s.tile([128, 1024], F32, tag="s", bufs=2)
                                    pscores.append(ps)
                                    for blk in range((wch + 511) // 512):
                                        w = min(512, wch - blk * 512)
                                        sl = slice(blk * 512, blk * 512 + w)
                                        nc.tensor.matmul(
                                            ps[:, sl], ident[:],
                                            bias_t[:, base + blk * 512: base + blk * 512 + w],
                                            start=True, stop=False,
                                        )
                                        nc.tensor.matmul(
                                            ps[:, sl], qt_slice(h, qi),
                                            kt_slice(h, base + blk * 512, w),
                                            start=False, stop=True,
                                        )
                                    nc.scalar.activation(
                                        att_t[:, base:base + wch], ps[:, 0:wch], AF.Exp,
                                        accum_out=dpart[:, ch:ch + 1],
                                    )
                                if nch == 2:
                                    dsum = smalls.tile([128, 1], F32, tag="dsum", bufs=4)
                                    nc.scalar.activation(dsum[:], dpart[:, 0:1], AF.Identity, bias=dpart[:, 1:2])
                                    dsum_ap = dsum[:]
                                else:
                                    dsum_ap = dpart[:, 0:1]
                                lnd = smalls.tile([128, 1], F32, tag="lnd", bufs=4)
                                nc.scalar.activation(lnd[:], dsum_ap, AF.Ln)
                                negl = smalls.tile([128, 1], F32, tag="negl", bufs=4)
                                nc.scalar.mul(negl[:], lnd[:], -1.0)
                                for ch in range(nch):
                                    base = ch * 1024
                                    wch = min(1024, W - base)
                                    nc.scalar.activation(
                                        att_t[:, base:base + wch], pscores[ch][:, 0:wch],
                                        AF.Exp, bias=negl[:],
                                    )
                                nc.sync.dma_start(
                                    attn_out[h, qi * 128:(qi + 1) * 128, 0:W],
                                    att_t[:, 0:W].bitcast(F32),
                                )
                                group_attn.append(att_t)

                            # ---- PV for this (h, gq) group ----
                            psc = ps_c.tile([64, 512], F32, tag="c", bufs=2)
                            for kt in range(4 * gq):  # full k rows
                                pst = ps_t.tile([128, 512], F32, tag="t", bufs=2)
                                for i in range(4):
                                    nc.tensor.matmul(
                                        pst[:, i * 128:(i + 1) * 128].bitcast(F32R),
                                        group_attn[i][:, kt * 128:(kt + 1) * 128],
                                        ident[:],
                                        is_transpose=True,
                                        start=(i == 0), stop=(i == 3),
                                    )
                                at = attp.tile([128, 512], F32R, tag="at", bufs=3)
                                nc.scalar.copy(at[:], pst[:])
                                nc.tensor.matmul(
                                    psc[:], v_slice(kt, h), at[:],
                                    start=(kt == 0), stop=False,
                                )
                            for ii in range(4):  # diagonal k rows
                                kt = 4 * gq + ii
                                pst = ps_t.tile([128, 512], F32, tag="t", bufs=2)
                                for i in range(ii, 4):
                                    nc.tensor.matmul(
                                        pst[:, i * 128:(i + 1) * 128].bitcast(F32R),
                                        group_attn[i][:, kt * 128:(kt + 1) * 128],
                                        ident[:],
                                        is_transpose=True,
                                        start=(i == ii), stop=(i == 3),
                                    )
                                at = attp.tile([128, 512], F32R, tag="at", bufs=3)
                                nc.scalar.copy(at[:, ii * 128:512], pst[:, ii * 128:512])
                                if ii == 0:
                                    nc.tensor.matmul(
                                        psc[:],
                                        v_slice(kt, h),
                                        at[:, 0:512],
                                        start=(kt == 0), stop=False,
                                    )
                                else:
                                    for i in range(ii, 4):
                                        nc.tensor.matmul(
                                            psc[:, i * 128:(i + 1) * 128],
                                            v_slice(kt, h),
                                            at[:, i * 128:(i + 1) * 128],
                                            start=False, stop=(ii == 3),
                                        )
                            nc.scalar.copy(
                                ctxT_sb[0:64, h * S + gq * 512:h * S + (gq + 1) * 512],
                                psc[:],
                            )

            # ---------------- output projection ----------------
            with (
                tc.tile_pool(name="po", bufs=1, space="PSUM") as po,
                tc.tile_pool(name="poutp", bufs=1) as poutp,
            ):
                with nc.named_scope("out_proj"):
                    for st in range(NS):
                        pout_t = poutp.tile([128, D], F32, tag="pout", bufs=2)
                        for ot in range(NO):
                            pso = po.tile([128, 512], F32, tag="o", bufs=4)
                            for h in range(HPC):
                                nc.tensor.matmul(
                                    pso[:],
                                    ctxT_sb[0:64, h * S + st * 128: h * S + (st + 1) * 128],
                                    wo_sb[0:64, h * D + ot * 512: h * D + (ot + 1) * 512],
                                    start=(h == 0), stop=(h == HPC - 1),
                                )
                            nc.scalar.copy(pout_t[:, ot * 512:(ot + 1) * 512], pso[:])
                        nc.sync.dma_start(pout[st * 128:(st + 1) * 128, :], pout_t[:])

    nc.compile()
    return nc


def _get_nc():
    key = (S, D, HPC)
    if key not in _nc_cache:
        _nc_cache[key] = _build(S, D, HPC)
    return _nc_cache[key]


def _prep_core_inputs(query, key, value, pos_bias, Wq, Wk, Wv, Wo):
    """Build the 8 per-core input maps (host-side prep)."""
    f = np.float32
    # causal mask folded into pos_bias: -1e30 strictly above the diagonal
    kk, qq = np.meshgrid(np.arange(S), np.arange(S))
    upper = kk > qq  # [q, k] True where masked
    pb_all = np.ascontiguousarray(pos_bias[0]).astype(f, copy=True)  # (H, S, S)
    pb_all[:, upper] = -1e30

    xT = {}
    for b in range(B):
        xT[("q", b)] = np.ascontiguousarray(query[b].T.astype(f))
        xT[("k", b)] = np.ascontiguousarray(key[b].T.astype(f))
        xT[("v", b)] = np.ascontiguousarray(value[b].T.astype(f))

    ident = np.eye(128, dtype=f)
    scale = np.float32(1.0 / np.sqrt(DK))

    in_maps = []
    for c in range(NCORES):
        b = c % 2
        g = c // 2
        j0 = g * HPC * DK
        j1 = j0 + HPC * DK
        in_maps.append({
            "xqT": xT[("q", b)],
            "xkT": xT[("k", b)],
            "xvT": xT[("v", b)],
            "wqT": np.ascontiguousarray(Wq[j0:j1, :].T.astype(f) * scale),
            "wkT": np.ascontiguousarray(Wk[j0:j1, :].T.astype(f)),
            "wvT": np.ascontiguousarray(Wv[j0:j1, :].T.astype(f)),
            "woT": np.ascontiguousarray(Wo[:, j0:j1].T.astype(f)),
            "pb": np.ascontiguousarray(pb_all[g * HPC:(g + 1) * HPC]),
            "ident": ident,
        })
    return in_maps


def _enable_jax_cache():
    try:
        import jax

        jax.config.update("jax_compilation_cache_dir", "/tmp/jax_kernel_cache")
        jax.config.update("jax_persistent_cache_min_entry_size_bytes", 0)
        jax.config.update("jax_persistent_cache_min_compile_time_secs", 2)
    except Exception:
        pass


def run_on_device(in_maps, trace=False):
    from concourse.bass_utils import run_bass_kernel_spmd

    _enable_jax_cache()
    nc = _get_nc()
    return run_bass_kernel_spmd(nc, in_maps, core_ids=list(range(NCORES)), trace=trace)


def kernel(query, key, value, pos_bias, mask, Wq, Wk, Wv, Wo, bo):
    query = np.asarray(query, dtype=np.float32)
    key = np.asarray(key, dtype=np.float32)
    value = np.asarray(value, dtype=np.float32)
    pos_bias = np.asarray(pos_bias, dtype=np.float32)
    Wq = np.asarray(Wq, dtype=np.float32)
    Wk = np.asarray(Wk, dtype=np.float32)
    Wv = np.asarray(Wv, dtype=np.float32)
    Wo = np.asarray(Wo, dtype=np.float32)
    bo = np.asarray(bo, dtype=np.float32)

    in_maps = _prep_core_inputs(query, key, value, pos_bias, Wq, Wk, Wv, Wo)
    res = run_on_device(in_maps)

    attn = np.empty((B, H, S, S), dtype=np.float32)
    output = np.zeros((B, S, D), dtype=np.float32)
    for c in range(NCORES):
        b = c % 2
        g = c // 2
        attn[b, g * HPC:(g + 1) * HPC] = res.results[c]["attn_out"]
        output[b] += res.results[c]["pout"]
    output += bo[None, None, :]
    return output, attn


# revision 11
# speedup vs baseline: 181.1923x; 181.1923x over previous
"""Bass/Trainium2 SPMD kernel for nn_MultiHeadAttention (B=2, S=2048, D=1024, H=16).

Sharding: 8 cores; core c handles batch b = c % 2 and 4 heads g = c // 2
(heads 4g..4g+3).  Each core computes its heads' Q/K/V projections, causal
softmax(QK^T/sqrt(dk) + pos_bias) with the mask folded into pos_bias on the
host (-1e30 above the diagonal), the attn output slice, and a partial output
projection (summed across head-groups on the host).

Device dataflow (all matmuls in float32r - full-speed fp32, ~1.5e-4 rel err):
  - host pre-transposes x so x^T loads are contiguous; weights pre-sliced,
    1/sqrt(dk) folded into Wq
  - projections produce Q^T,K^T [j, s] and V [s, j] directly on PE
  - scores: pos_bias loaded to PSUM via identity-matmul, QK^T accumulates
  - softmax: exp pass 1 on ScalarE with fused row-sum (accum_out), then
    pass 2 recomputes exp(score - ln(rowsum)) => normalized attn, no DVE
    reciprocal/reduce needed (those ops are broken on this runtime)
  - attn tiles PE-transposed for the PV matmul; context^T @ Wo^T partial out
"""
import numpy as np

B, S, D, H = 2, 2048, 1024, 16
DK = 64
NCORES = 8
HPC = 4  # heads per core

_nc_cache = {}


def _build(S, D, HPC, write_zeros=True):
    import concourse.bacc as bacc
    import concourse.mybir as mybir
    import concourse.tile as tile

    F32 = mybir.dt.float32
    F32R = mybir.dt.float32r
    AF = mybir.ActivationFunctionType

    JW = HPC * DK          # per-core j width (256)
    NJT = max(1, JW // 128)  # j partition-tiles (2)
    ND = D // 128          # d tiles (8)
    NS = S // 128          # s tiles (16)
    NG = NS // 4           # q groups of 4 tiles (4)
    NO = D // 512          # out col blocks (2)

    nc = bacc.Bacc("TRN2", target_bir_lowering=False, debug=False)

    xqT = nc.dram_tensor("xqT", [D, S], F32R, kind="ExternalInput").ap()
    xkT = nc.dram_tensor("xkT", [D, S], F32R, kind="ExternalInput").ap()
    xvT = nc.dram_tensor("xvT", [D, S], F32R, kind="ExternalInput").ap()
    wqT = nc.dram_tensor("wqT", [D, JW], F32R, kind="ExternalInput").ap()
    wkT = nc.dram_tensor("wkT", [D, JW], F32R, kind="ExternalInput").ap()
    wvT = nc.dram_tensor("wvT", [D, JW], F32R, kind="ExternalInput").ap()
    woT = nc.dram_tensor("woT", [JW, D], F32R, kind="ExternalInput").ap()
    pb = nc.dram_tensor("pb", [HPC, S, S], F32R, kind="ExternalInput").ap()
    identD = nc.dram_tensor("ident", [128, 128], F32R, kind="ExternalInput").ap()

    attn_out = nc.dram_tensor("attn_out", [HPC, S, S], F32, kind="ExternalOutput").ap()
    pout = nc.dram_tensor("pout", [S, D], F32, kind="ExternalOutput").ap()

    with tile.TileContext(nc) as tc:
        with (
            tc.tile_pool(name="const", bufs=1) as cpool,
            tc.tile_pool(name="qkv", bufs=1) as qkv,
        ):
            ident = cpool.tile([128, 128], F32R)
            nc.sync.dma_start(ident[:], identD[:])
            # wo laid per-head on partitions 0..63: col h*D + o
            wo_sb = cpool.tile([128, HPC * D], F32R)
            for h in range(HPC):
                nc.sync.dma_start(wo_sb[0:64, h * D:(h + 1) * D], woT[h * 64:(h + 1) * 64, :])
            zero_sb = cpool.tile([128, S], F32)
            nc.vector.memset(zero_sb[:], 0.0)

            # upper-triangle zeros of attn, written up front (overlap proj DMA)
            if write_zeros:
                for h in range(HPC):
                    for qi in range(NS - 1):
                        W = (qi + 1) * 128
                        nc.gpsimd.dma_start(
                            attn_out[h, qi * 128:(qi + 1) * 128, W:S],
                            zero_sb[:, 0:S - W],
                        )

            QT_sb = qkv.tile([128, NJT * S], F32R)
            KT_sb = qkv.tile([128, NJT * S], F32R)
            V_sb = qkv.tile([128, NS * JW], F32R)
            # per-head ctx^T on partitions 0..63: col h*S + s
            ctxT_sb = qkv.tile([128, HPC * S], F32R)

            # ---------------- projections ----------------
            with (
                tc.tile_pool(name="xt", bufs=1) as xtp,
                tc.tile_pool(name="wts", bufs=1) as wts,
                tc.tile_pool(name="pp", bufs=1, space="PSUM") as pp,
            ):
                wq_sb = wts.tile([128, ND * JW], F32R)
                wk_sb = wts.tile([128, ND * JW], F32R)
                wv_sb = wts.tile([128, ND * JW], F32R)
                for d in range(ND):
                    nc.sync.dma_start(wq_sb[:, d * JW:(d + 1) * JW], wqT[d * 128:(d + 1) * 128, :])
                    nc.sync.dma_start(wk_sb[:, d * JW:(d + 1) * JW], wkT[d * 128:(d + 1) * 128, :])
                    nc.sync.dma_start(wv_sb[:, d * JW:(d + 1) * JW], wvT[d * 128:(d + 1) * 128, :])
                NSB = S // 512
                with nc.named_scope("proj_qk"):
                    for (xT, w_sb, O_sb) in ((xqT, wq_sb, QT_sb), (xkT, wk_sb, KT_sb)):
                        psums = [pp.tile([128, 512], F32, tag="pj", bufs=8, name=f"pq{_i}") for _i in range(NJT * NSB)]
                        for d in range(ND):
                            xt = xtp.tile([128, S], F32R, tag="xt", bufs=2)
                            nc.sync.dma_start(xt[:], xT[d * 128:(d + 1) * 128, :])
                            for sb4 in range(NSB):
                                for jt in range(NJT):
                                    nc.tensor.matmul(
                                        psums[sb4 * NJT + jt][:],
                                        w_sb[:, d * JW + jt * 128: d * JW + (jt + 1) * 128],
                                        xt[:, sb4 * 512:(sb4 + 1) * 512],
                                        start=(d == 0),
                                        stop=(d == ND - 1),
                                    )
                        for sb4 in range(NSB):
                            for jt in range(NJT):
                                nc.scalar.copy(
                                    O_sb[:, jt * S + sb4 * 512: jt * S + (sb4 + 1) * 512],
                                    psums[sb4 * NJT + jt][:],
                                )
                with nc.named_scope("proj_v"):
                    nhalf = 2 if NSB >= 2 else 1
                    for half in range(nhalf):
                        sbs = list(range(half * NSB // nhalf, (half + 1) * NSB // nhalf))
                        psums = [pp.tile([128, 512], F32, tag="pj", bufs=8, name=f"pv{_i}") for _i in range(4 * len(sbs))]
                        for d in range(ND):
                            xt = xtp.tile([128, S], F32R, tag="xt", bufs=2)
                            nc.sync.dma_start(
                                xt[:, 0:len(sbs) * 512],
                                xvT[d * 128:(d + 1) * 128, sbs[0] * 512:(sbs[-1] + 1) * 512],
                            )
                            for si, sb4 in enumerate(sbs):
                                for st in range(4):
                                    nc.tensor.matmul(
                                        psums[si * 4 + st][:, 0:JW],
                                        xt[:, (si * 4 + st) * 128:(si * 4 + st + 1) * 128],
                                        wv_sb[:, d * JW:(d + 1) * JW],
                                        start=(d == 0),
                                        stop=(d == ND - 1),
                                    )
                        for si, sb4 in enumerate(sbs):
                            for st in range(4):
                                nc.scalar.copy(
                                    V_sb[:, (sb4 * 4 + st) * JW:(sb4 * 4 + st + 1) * JW],
                                    psums[si * 4 + st][:, 0:JW],
                                )

            # ---------------- attention ----------------
            def qt_slice(h, qi):
                return QT_sb[(h % 2) * 64:(h % 2) * 64 + 64,
                             (h // 2) * S + qi * 128:(h // 2) * S + (qi + 1) * 128]

            def kt_slice(h, k0, w):
                return KT_sb[(h % 2) * 64:(h % 2) * 64 + 64,
                             (h // 2) * S + k0:(h // 2) * S + k0 + w]

            def v_slice(kt, h):
                return V_sb[:, kt * JW + h * DK: kt * JW + h * DK + DK]

            with (
                tc.tile_pool(name="att", bufs=1) as attp,
                tc.tile_pool(name="smalls", bufs=1) as smalls,
                tc.tile_pool(name="ps_s", bufs=1, space="PSUM") as ps_s,
                tc.tile_pool(name="ps_t", bufs=1, space="PSUM") as ps_t,
                tc.tile_pool(name="ps_c", bufs=1, space="PSUM") as ps_c,
            ):
                with nc.named_scope("attention"):
                    for h in range(HPC):
                        hrow = (h % 2) * 64
                        for gq in range(NG):
                            group_attn = []
                            for qi4 in range(4):
                                qi = gq * 4 + qi4
                                W = (qi + 1) * 128
                                nch = (W + 1023) // 1024
                                bias_t = attp.tile([128, S], F32R, tag="bias", bufs=2)
                                nc.sync.dma_start(
                                    bias_t[:, 0:W], pb[h, qi * 128:(qi + 1) * 128, 0:W]
                                )
                                att_t = attp.tile([128, S], F32R, tag="attn", bufs=6)
                                dpart = smalls.tile([128, 2], F32, tag="dpart", bufs=4)
                                pscores = []
                                for ch in range(nch):
                                    base = ch * 1024
                                    wch = min(1024, W - base)
                                    ps = ps_s.tile([128, 1024], F32, tag="s", bufs=2)
                                    pscores.append(ps)
                                    for blk in range((wch + 511) // 512):
                                        w = min(512, wch - blk * 512)
                                        sl = slice(blk * 512, blk * 512 + w)
                                        nc.tensor.matmul(
                                            ps[:, sl], ident[:],
                                            bias_t[:, base + blk * 512: base + blk * 512 + w],
                                            start=True, stop=False,
                                        )
                                        nc.tensor.matmul(
                                            ps[:, sl], qt_slice(h, qi),
                                            kt_slice(h, base + blk * 512, w),
                                            start=False, stop=True,
                                        )
                                    nc.scalar.activation(
                                        att_t[:, base:base + wch], ps[:, 0:wch], AF.Exp,
                                        accum_out=dpart[:, ch:ch + 1],
                                    )
                                group_attn.append((att_t, dpart, W, nch))

                            # ---- batched normalization for the group ----
                            lnds = []
                            for (att_t, dpart, W, nch) in group_attn:
                                if nch == 2:
                                    dsum = smalls.tile([128, 1], F32, tag="dsum", bufs=8)
                                    nc.scalar.activation(dsum[:], dpart[:, 0:1], AF.Identity, bias=dpart[:, 1:2])
                                    dsum_ap = dsum[:]
                                else:
                                    dsum_ap = dpart[:, 0:1]
                                lnd = smalls.tile([128, 1], F32, tag="lnd", bufs=8)
                                nc.scalar.activation(lnd[:], dsum_ap, AF.Ln)
                                lnds.append(lnd)
                            recips = []
                            for lnd in lnds:
                                recip = smalls.tile([128, 1], F32, tag="recip", bufs=8)
                                nc.scalar.activation(recip[:], lnd[:], AF.Exp, scale=-1.0)
                                recips.append(recip)
                            for qi4, ((att_t, dpart, W, nch), recip) in enumerate(zip(group_attn, recips)):
                                qi = gq * 4 + qi4
                                nc.scalar.mul(att_t[:, 0:W], att_t[:, 0:W], recip[:])
                                nc.gpsimd.dma_start(
                                    attn_out[h, qi * 128:(qi + 1) * 128, 0:W],
                                    att_t[:, 0:W].bitcast(F32),
                                )
                            group_attn = [t[0] for t in group_attn]

                            # ---- PV for this (h, gq) group ----
                            psc = ps_c.tile([64, 512], F32, tag="c", bufs=2)
                            for kt in range(4 * gq):  # full k rows
                                pst = ps_t.tile([128, 512], F32, tag="t", bufs=2)
                                for i in range(4):
                                    nc.tensor.matmul(
                                        pst[:, i * 128:(i + 1) * 128].bitcast(F32R),
                                        group_attn[i][:, kt * 128:(kt + 1) * 128],
                                        ident[:],
                                        is_transpose=True,
                                        start=(i == 0), stop=(i == 3),
                                    )
                                at = attp.tile([128, 512], F32R, tag="at", bufs=3)
                                nc.vector.tensor_copy(at[:], pst[:])
                                nc.tensor.matmul(
                                    psc[:], v_slice(kt, h), at[:],
                                    start=(kt == 0), stop=False,
                                )
                            for ii in range(4):  # diagonal k rows
                                kt = 4 * gq + ii
                                pst = ps_t.tile([128, 512], F32, tag="t", bufs=2)
                                for i in range(ii, 4):
                                    nc.tensor.matmul(
                                        pst[:, i * 128:(i + 1) * 128].bitcast(F32R),
                                        group_attn[i][:, kt * 128:(kt + 1) * 128],
                                        ident[:],
                                        is_transpose=True,
                                        start=(i == ii), stop=(i == 3),
                                    )
                                at = attp.tile([128, 512], F32R, tag="at", bufs=3)
                                nc.vector.tensor_copy(at[:, ii * 128:512], pst[:, ii * 128:512])
                                if ii == 0:
                                    nc.tensor.matmul(
                                        psc[:],
                                        v_slice(kt, h),
                                        at[:, 0:512],
                                        start=(kt == 0), stop=False,
                                    )
                                else:
                                    for i in range(ii, 4):
                                        nc.tensor.matmul(
                                            psc[:, i * 128:(i + 1) * 128],
                                            v_slice(kt, h),
                                            at[:, i * 128:(i + 1) * 128],
                                            start=False, stop=(ii == 3),
                                        )
                            nc.scalar.copy(
                                ctxT_sb[0:64, h * S + gq * 512:h * S + (gq + 1) * 512],
                                psc[:],
                            )

            # ---------------- output projection ----------------
            with (
                tc.tile_pool(name="po", bufs=1, space="PSUM") as po,
                tc.tile_pool(name="poutp", bufs=1) as poutp,
            ):
                with nc.named_scope("out_proj"):
                    for st in range(NS):
                        pout_t = poutp.tile([128, D], F32, tag="pout", bufs=2)
                        for ot in range(NO):
                            pso = po.tile([128, 512], F32, tag="o", bufs=4)
                            for h in range(HPC):
                                nc.tensor.matmul(
                                    pso[:],
                                    ctxT_sb[0:64, h * S + st * 128: h * S + (st + 1) * 128],
                                    wo_sb[0:64, h * D + ot * 512: h * D + (ot + 1) * 512],
                                    start=(h == 0), stop=(h == HPC - 1),
                                )
                            nc.scalar.copy(pout_t[:, ot * 512:(ot + 1) * 512], pso[:])
                        nc.sync.dma_start(pout[st * 128:(st + 1) * 128, :], pout_t[:])

    nc.compile()
    return nc


def _get_nc():
    key = (S, D, HPC)
    if key not in _nc_cache:
        _nc_cache[key] = _build(S, D, HPC)
    return _nc_cache[key]


def _prep_core_inputs(query, key, value, pos_bias, Wq, Wk, Wv, Wo):
    """Build the 8 per-core input maps (host-side prep)."""
    f = np.float32
    # causal mask folded into pos_bias: -1e30 strictly above the diagonal
    kk, qq = np.meshgrid(np.arange(S), np.arange(S))
    upper = kk > qq  # [q, k] True where masked
    pb_all = np.ascontiguousarray(pos_bias[0]).astype(f, copy=True)  # (H, S, S)
    pb_all[:, upper] = -1e30

    xT = {}
    for b in range(B):
        xT[("q", b)] = np.ascontiguousarray(query[b].T.astype(f))
        xT[("k", b)] = np.ascontiguousarray(key[b].T.astype(f))
        xT[("v", b)] = np.ascontiguousarray(value[b].T.astype(f))

    ident = np.eye(128, dtype=f)
    scale = np.float32(1.0 / np.sqrt(DK))

    in_maps = []
    for c in range(NCORES):
        b = c % 2
        g = c // 2
        j0 = g * HPC * DK
        j1 = j0 + HPC * DK
        in_maps.append({
            "xqT": xT[("q", b)],
            "xkT": xT[("k", b)],
            "xvT": xT[("v", b)],
            "wqT": np.ascontiguousarray(Wq[j0:j1, :].T.astype(f) * scale),
            "wkT": np.ascontiguousarray(Wk[j0:j1, :].T.astype(f)),
            "wvT": np.ascontiguousarray(Wv[j0:j1, :].T.astype(f)),
            "woT": np.ascontiguousarray(Wo[:, j0:j1].T.astype(f)),
            "pb": np.ascontiguousarray(pb_all[g * HPC:(g + 1) * HPC]),
            "ident": ident,
        })
    return in_maps


def _enable_jax_cache():
    try:
        import jax

        jax.config.update("jax_compilation_cache_dir", "/tmp/jax_kernel_cache")
        jax.config.update("jax_persistent_cache_min_entry_size_bytes", 0)
        jax.config.update("jax_persistent_cache_min_compile_time_secs", 2)
    except Exception:
        pass


def run_on_device(in_maps, trace=False):
    from concourse.bass_utils import run_bass_kernel_spmd

    _enable_jax_cache()
    nc = _get_nc()
    return run_bass_kernel_spmd(nc, in_maps, core_ids=list(range(NCORES)), trace=trace)


def kernel(query, key, value, pos_bias, mask, Wq, Wk, Wv, Wo, bo):
    query = np.asarray(query, dtype=np.float32)
    key = np.asarray(key, dtype=np.float32)
    value = np.asarray(value, dtype=np.float32)
    pos_bias = np.asarray(pos_bias, dtype=np.float32)
    Wq = np.asarray(Wq, dtype=np.float32)
    Wk = np.asarray(Wk, dtype=np.float32)
    Wv = np.asarray(Wv, dtype=np.float32)
    Wo = np.asarray(Wo, dtype=np.float32)
    bo = np.asarray(bo, dtype=np.float32)

    in_maps = _prep_core_inputs(query, key, value, pos_bias, Wq, Wk, Wv, Wo)
    res = run_on_device(in_maps)

    attn = np.empty((B, H, S, S), dtype=np.float32)
    output = np.zeros((B, S, D), dtype=np.float32)
    for c in range(NCORES):
        b = c % 2
        g = c // 2
        attn[b, g * HPC:(g + 1) * HPC] = res.results[c]["attn_out"]
        output[b] += res.results[c]["pout"]
    output += bo[None, None, :]
    return output, attn
